# revision 21
# baseline (speedup 1.0000x reference)
"""Trainium2 Bass kernel for the ragged text-CNN problem.

Math: for tokens x[t,b] with embedding tables E,U [V,D] and conv
weights w [H, 2D, 2] (kernel size 2 over time):

    conv[b,h,t] = w0_h . e_{t,b} + w1_h . e_{t+1,b} + cb_h
    scores      = (max over valid t of conv) @ out_w.T + out_b

where e = concat(E[tok], U[tok]).  We precompute a fused table

    T[v, 0:64]   = concat(E[v],U[v]) . w0_h        (the "A" half)
    T[v, 64:128] = concat(E[v],U[v]) . w1_h        (the "B" half)

so conv[b,h,t] = T[tok_t, h] + T[tok_{t+1}, 64+h].  The ragged mask is
free: PAD (=1) appears exactly at positions t >= len, so forcing
T[1, 0:64] = -1e30 makes every masked conv position -1e30.

Distribution over 8 cores: phase A (table build) is vocab-sharded
(V/8 rows each) and exchanged with an AllGather; phase B (gather +
conv + masked max + linear head) is batch-sharded (B/8 sentences).
"""

import numpy as np

try:
    import concourse.bass as bass
except ImportError:  # harness runs from a bare directory
    import sys

    sys.path.insert(0, "/opt/trn_rl_repo")
    import concourse.bass as bass

import concourse.bass_isa as bass_isa
import concourse.mybir as mybir
from concourse.bacc import Bacc
import concourse.tile as tile
from concourse.bass_utils import run_bass_kernel_spmd
from concourse.masks import make_identity

V, D, H, S, B = 50000, 300, 64, 512, 256
NCORES = 8
VS = V // NCORES        # vocab rows per core (6250)
BS = B // NCORES        # sentences per core (32)
F = 2 * H               # fused feature width (128)
NEG = -1.0e30
P = 128

F32 = mybir.dt.float32
BF16 = mybir.dt.bfloat16
I32 = mybir.dt.int32


def build_nc(v=V, d=D, h=H, s=S, bs=BS, ncores=NCORES, mm_bf16=True, debug_probe=False):
    """Build the per-core SPMD Bass program (identical on all cores)."""
    vs = v // ncores
    f = 2 * h
    kb = s // P
    assert s % P == 0 and v % ncores == 0
    fh = h * bs
    n_vt = (vs + P - 1) // P
    chunks = []  # (src_is_u, col0, width) over the 2D concat dim
    for base in range(0, d, P):
        chunks.append((False, base, min(P, d - base)))
    for base in range(0, d, P):
        chunks.append((True, base, min(P, d - base)))
    nch = len(chunks)
    mm_dt = BF16 if mm_bf16 else F32

    nc = Bacc()
    e_sh = nc.dram_tensor("e_shard", [vs, d], F32, kind="ExternalInput")
    u_sh = nc.dram_tensor("u_shard", [vs, d], F32, kind="ExternalInput")
    sent = nc.dram_tensor("sent", [s, bs], I32, kind="ExternalInput")
    sent2 = nc.dram_tensor("sent2", [s, bs], I32, kind="ExternalInput")
    convw = nc.dram_tensor("conv_w", [h, d * 2, 2], F32, kind="ExternalInput")
    convb = nc.dram_tensor("conv_b", [1, h], F32, kind="ExternalInput")
    outw = nc.dram_tensor("out_w", [2, h], F32, kind="ExternalInput")
    outb = nc.dram_tensor("out_b", [1, 2], F32, kind="ExternalInput")
    patch = nc.dram_tensor("patch", [2, f], F32, kind="ExternalInput")
    t_shard = nc.dram_tensor("t_shard", [vs, f], F32)
    t_full = nc.dram_tensor("t_full", [v + 1, f], F32, addr_space="Shared")
    scores = nc.dram_tensor("scores", [bs, 2], F32, kind="ExternalOutput")
    if debug_probe:
        tsh_out = nc.dram_tensor("tsh_out", [vs, f], F32, kind="ExternalOutput")
        tf_out = nc.dram_tensor("tf_out", [v, f], F32, kind="ExternalOutput")
        ga_out = nc.dram_tensor("ga_out", [P, kb * bs * h], F32, kind="ExternalOutput")
        gb_out = nc.dram_tensor("gb_out", [P, kb * bs * h], F32, kind="ExternalOutput")

    with tile.TileContext(nc) as tc:
        with tc.tile_pool(name="const", bufs=1) as cpool:
            ident = cpool.tile([P, P], F32, tag="identf")
            make_identity(nc, ident[:])
            identb = cpool.tile([P, P], mm_dt, tag="identb")
            if mm_bf16:
                make_identity(nc, identb[:])

            # ---- W2 prep: w2c[ci][dd, k*h + hh] = conv_w[hh, c0+dd, k]
            cw_sb = cpool.tile([h, d * 2 * 2], F32, tag="cw")
            nc.sync.dma_start(
                cw_sb[:], convw[:, :, :].rearrange("a b c -> a (b c)")
            )
            cw_v = cw_sb[:].rearrange("a (b c) -> a b c", c=2)
            w2cs = []
            with tc.tile_pool(name="w2psum", bufs=2, space="PSUM") as wpp:
                for ci, (_, c0, dc) in enumerate(chunks):
                    w2c = cpool.tile([P, f], mm_dt, tag=f"w2_{ci}")
                    w2cs.append(w2c)
                    cglob = c0 + (d if chunks[ci][0] else 0)
                    for k in range(2):
                        tp = wpp.tile([P, h], F32, tag="wtp")
                        nc.tensor.transpose(
                            tp[:dc, :h],
                            cw_v[:, cglob : cglob + dc, k],
                            ident[:h, :h],
                        )
                        nc.vector.tensor_copy(
                            w2c[:dc, k * h : (k + 1) * h], tp[:dc, :h]
                        )

            patch_sb = cpool.tile([2, f], F32, tag="patch")
            nc.sync.dma_start(patch_sb[:], patch[:, :])

            # ---- Phase A: T_shard tiles
            with (
                tc.tile_pool(name="pa", bufs=3) as papool,
                tc.tile_pool(name="pa_ps", bufs=3, space="PSUM") as papsum,
                tc.tile_pool(name="pa_acc", bufs=2, space="PSUM") as paacc,
            ):
                for ti in range(n_vt):
                    r0 = ti * P
                    nr = min(P, vs - r0)
                    e_t = papool.tile([P, d], mm_dt, tag="e_t")
                    u_t = papool.tile([P, d], mm_dt, tag="u_t")
                    if mm_bf16:
                        nc.gpsimd.dma_start(e_t[:nr], e_sh[r0 : r0 + nr, :])
                        nc.gpsimd.dma_start(u_t[:nr], u_sh[r0 : r0 + nr, :])
                    else:
                        nc.sync.dma_start(e_t[:nr], e_sh[r0 : r0 + nr, :])
                        nc.sync.dma_start(u_t[:nr], u_sh[r0 : r0 + nr, :])
                    ets = papool.tile([P, nch * P], mm_dt, tag="ets")
                    for ci, (is_u, c0, dc) in enumerate(chunks):
                        src = u_t if is_u else e_t
                        tp = papsum.tile([P, P], mm_dt, tag="tp")
                        nc.tensor.transpose(
                            tp[:dc, :nr],
                            src[:nr, c0 : c0 + dc],
                            identb[:nr, :nr] if mm_bf16 else ident[:nr, :nr],
                        )
                        nc.vector.tensor_copy(
                            ets[:dc, ci * P : ci * P + nr], tp[:dc, :nr]
                        )
                    acc = paacc.tile([P, f], F32, tag="acc")
                    for ci, (is_u, c0, dc) in enumerate(chunks):
                        nc.tensor.matmul(
                            acc[:nr, :],
                            lhsT=ets[:dc, ci * P : ci * P + nr],
                            rhs=w2cs[ci][:dc, :],
                            start=(ci == 0),
                            stop=(ci == nch - 1),
                        )
                    t_sb = papool.tile([P, f], F32, tag="t_sb")
                    nc.vector.tensor_copy(t_sb[:nr], acc[:nr])
                    if ti == 0:
                        # core 0's patch is -1e30 on the A half; others zero
                        nc.vector.tensor_add(
                            t_sb[0:2, :], t_sb[0:2, :], patch_sb[0:2, :]
                        )
                    nc.sync.dma_start(t_shard[r0 : r0 + nr, :], t_sb[:nr])

            # ---- exchange shards
            nc.gpsimd.collective_compute(
                "AllGather",
                mybir.AluOpType.bypass,
                replica_groups=[list(range(ncores))],
                ins=[t_shard[:, :]],
                outs=[t_full[0:v, :]],
            )

            if debug_probe:
                nc.sync.dma_start(tsh_out[:, :], t_shard[:, :])
                nc.sync.dma_start(tf_out[:, :], t_full[:, :])

            neg_sb = cpool.tile([1, f], F32, tag="negrow")
            nc.vector.memset(neg_sb[:], NEG)
            nc.sync.dma_start(t_full[v : v + 1, :], neg_sb[:])

            # ---- Phase B: gather + conv + masked max + head
            with (
                tc.tile_pool(name="pb", bufs=1) as pbpool,
                tc.tile_pool(name="pbh", bufs=1) as hpool,
                tc.tile_pool(name="pb_ps", bufs=4, space="PSUM") as pbpsum,
            ):
                # --- token gather via dma_gather (int16 idx; split table)
                # idx order i = b*s + t  ->  out[p=i%128, j=i//128] with
                # j = b*kb + k, t = k*128 + p.
                nidx = s * bs
                nw = nidx // 16
                nj = nidx // P
                split = 32768 if v > 32768 else (v + 1) // 2
                # wrapped idx layout: idx i=b*s+t at (partition i%16, col
                # i//16) = (t%16, b*(s//16)+t//16); the queue-q gather reads
                # partitions [32q, 32q+32), so replicate the wrap into each
                # group a call needs: swa groups 0-3 (queues 0,1), swb
                # groups 4-7 (queues 2,3).
                def load_wrapped(dst, src_dram, groups):
                    for g in groups:
                        nc.sync.dma_start(
                            dst[16 * g : 16 * (g + 1), :],
                            bass.AP(
                                src_dram,
                                0,
                                [[bs, 16], [1, bs], [16 * bs, s // 16]],
                            ),
                        )

                swa = pbpool.tile([P, nw], I32, tag="swa")
                swb = pbpool.tile([P, nw], I32, tag="swb")
                load_wrapped(swa, sent, range(0, 2))
                load_wrapped(swb, sent2, range(0, 2))
                ilo_a = pbpool.tile([P, nw], mybir.dt.int16, tag="ilo_a")
                ihi_a = pbpool.tile([P, nw], mybir.dt.int16, tag="ihi_a")
                ilo_b = pbpool.tile([P, nw], mybir.dt.int16, tag="ilo_b")
                ihi_b = pbpool.tile([P, nw], mybir.dt.int16, tag="ihi_b")
                # queue-0 cores read idxs from partitions [0,16) and [16,32)
                # ilo = tok < split ? tok : 0        (row 0 = -1e30)
                # ihi = tok >= split ? tok-split : v-split   (row v = -1e30)
                for t in (ilo_a, ihi_a, ilo_b, ihi_b):
                    nc.vector.memset(t[:], 0)
                c2 = pbpool.tile([P, nw], I32, tag="c2")
                c1 = pbpool.tile([P, nw], I32, tag="c1")
                d2 = pbpool.tile([P, nw], I32, tag="d2")
                for sw, ilo, ihi in ((swa, ilo_a, ihi_a), (swb, ilo_b, ihi_b)):
                    nc.vector.tensor_scalar(
                        c2[0:32], sw[0:32], split, None, mybir.AluOpType.is_ge
                    )
                    nc.vector.tensor_scalar(
                        c1[0:32], sw[0:32], split, None, mybir.AluOpType.is_lt
                    )
                    nc.vector.tensor_tensor(
                        ilo[0:32], sw[0:32], c1[0:32], op=mybir.AluOpType.mult
                    )
                    nc.vector.tensor_scalar(
                        d2[0:32], sw[0:32], v, None, mybir.AluOpType.subtract
                    )
                    nc.vector.tensor_tensor(
                        d2[0:32], d2[0:32], c2[0:32], op=mybir.AluOpType.mult
                    )
                    nc.vector.tensor_scalar(
                        ihi[0:32], d2[0:32], v - split, None, mybir.AluOpType.add
                    )
                ga = pbpool.tile([P, nj * h], F32, tag="ga")
                gah = pbpool.tile([P, nj * h], F32, tag="gah")
                gb = pbpool.tile([P, nj * h], F32, tag="gb")
                gbh = pbpool.tile([P, nj * h], F32, tag="gbh")
                gathers = [
                    (ga, t_full[0:split, 0:h], ilo_a, 0),
                    (gah, t_full[split : v + 1, 0:h], ihi_a, 0),
                    (gb, t_full[0:split, h:f], ilo_b, 0),
                    (gbh, t_full[split : v + 1, h:f], ihi_b, 0),
                ]
                # ring carveout holds 2048 descs/direction; one call may
                # carry at most ~16k idxs (descs = nidx/16 + 1), so chunk.
                max_chunk = 8192
                chunks_i = []
                i0 = 0
                while i0 < nidx:
                    cn = min(max_chunk, nidx - i0)
                    chunks_i.append((i0, cn))
                    i0 += cn
                for out_t, in_ap, idx_t, q in gathers:
                    ov = out_t[:].rearrange("p (j c) -> p j c", c=h)
                    for i0, cn in chunks_i:
                        nc.gpsimd.dma_gather(
                            out_ap=ov[:, i0 // P : (i0 + cn) // P, :],
                            in_ap=in_ap,
                            idxs_ap=idx_t[:, i0 // 16 : (i0 + cn) // 16],
                            num_idxs=cn,
                            num_idxs_reg=cn,
                            elem_size=h,
                            elem_step=f,
                            queue_num=q,
                            single_packet=False,
                        )
                # merge: wrong-table entries are -1e30, so max picks
                # the real row
                nc.any.tensor_max(ga[:], ga[:], gah[:])
                nc.any.tensor_max(gb[:], gb[:], gbh[:])
                # conv[p, (b, k, c)] = ga + gb  (written in place of gb)
                nc.vector.tensor_add(gb[:], ga[:], gb[:])
                c4 = gb[:].rearrange("p (b k c) -> p b k c", b=bs, k=kb)
                # max over k blocks -> m [p, (b, h)]
                if kb > 1:
                    m = hpool.tile([P, fh], F32, tag="m")
                    nc.any.tensor_max(m[:], c4[:, :, 0, :], c4[:, :, 1, :])
                    for k in range(2, kb):
                        nc.any.tensor_max(m[:], m[:], c4[:, :, k, :])
                    m_ap = m[:]
                else:
                    m_ap = c4[:, :, 0, :]
                # per-sentence: transpose [128 tok, 64 feat] -> [64, 128] and
                # reduce over the 128 tokens, into pooled_t[:, b]
                pooled_t = pbpool.tile([h + 1, bs], F32, tag="pooled_t")
                nc.vector.memset(pooled_t[h : h + 1, :], 1.0)
                for b in range(bs):
                    mt = pbpsum.tile([h, P], F32, tag="mt")
                    msl = (
                        m[:, b * h : (b + 1) * h]
                        if kb > 1
                        else c4[:, b, 0, :]
                    )
                    nc.tensor.transpose(mt[:, :], msl, ident[:, :])
                    nc.vector.reduce_max(
                        pooled_t[0:h, b : b + 1],
                        mt[:, :],
                        axis=mybir.AxisListType.X,
                    )
                cb_t = pbpool.tile([h, 1], F32, tag="cb_t")
                nc.sync.dma_start(cb_t[:, :], convb[:, :].rearrange("o c -> c o"))
                nc.vector.tensor_scalar_add(
                    pooled_t[0:h, :], pooled_t[0:h, :], cb_t[:, :]
                )
                ow_t = pbpool.tile([h + 1, 2], F32, tag="ow_t")
                nc.sync.dma_start(ow_t[0:h, :], outw[:, :].rearrange("a c -> c a"))
                nc.sync.dma_start(ow_t[h : h + 1, :], outb[:, :])
                sc_ps = pbpsum.tile([bs, 2], F32, tag="sc")
                nc.tensor.matmul(
                    sc_ps[:, :],
                    lhsT=pooled_t[:, :],
                    rhs=ow_t[:, :],
                    start=True,
                    stop=True,
                )
                sc_sb = pbpool.tile([bs, 2], F32, tag="sc_sb")
                nc.vector.tensor_copy(sc_sb[:], sc_ps[:])
                nc.sync.dma_start(scores[:, :], sc_sb[:])

    nc.finalize()
    return nc


_NC_CACHE = {}


def _get_nc():
    if "nc" not in _NC_CACHE:
        _NC_CACHE["nc"] = build_nc()
    return _NC_CACHE["nc"]


def make_in_maps(sentences, E, U, conv_w, conv_b, out_w, out_b,
                 v=V, h=H, ncores=NCORES):
    vs = v // ncores
    bs = sentences.shape[1] // ncores
    f = 2 * h
    sent_shift = np.concatenate(
        [sentences[1:], np.zeros((1, sentences.shape[1]), np.int32)], axis=0
    )
    in_maps = []
    for c in range(ncores):
        pt = np.zeros((2, f), np.float32)
        if c == 0:
            pt[0, :] = NEG
            pt[1, :h] = NEG
        in_maps.append(
            {
                "e_shard": np.ascontiguousarray(E[c * vs : (c + 1) * vs]),
                "u_shard": np.ascontiguousarray(U[c * vs : (c + 1) * vs]),
                "sent": np.ascontiguousarray(
                    sentences[:, c * bs : (c + 1) * bs]
                ),
                "sent2": np.ascontiguousarray(
                    sent_shift[:, c * bs : (c + 1) * bs]
                ),
                "conv_w": conv_w,
                "conv_b": conv_b.reshape(1, h),
                "out_w": out_w,
                "out_b": out_b.reshape(1, 2),
                "patch": pt,
            }
        )
    return in_maps


def kernel(sentences, E, U, conv_w, conv_b, out_w, out_b):
    sentences = np.asarray(sentences, dtype=np.int32)
    E = np.asarray(E, dtype=np.float32)
    U = np.asarray(U, dtype=np.float32)
    conv_w = np.asarray(conv_w, dtype=np.float32)
    conv_b = np.asarray(conv_b, dtype=np.float32)
    out_w = np.asarray(out_w, dtype=np.float32)
    out_b = np.asarray(out_b, dtype=np.float32)

    nc = _get_nc()
    in_maps = make_in_maps(sentences, E, U, conv_w, conv_b, out_w, out_b)
    res = run_bass_kernel_spmd(nc, in_maps, list(range(NCORES)))
    return np.concatenate(
        [res.results[c]["scores"] for c in range(NCORES)], axis=0
    )


# revision 25
# speedup vs baseline: 145.8178x; 145.8178x over previous
"""Trainium2 Bass kernel for the ragged text-CNN problem.

Math: for tokens x[t,b] with embedding tables E,U [V,D] and conv
weights w [H, 2D, 2] (kernel size 2 over time):

    conv[b,h,t] = w0_h . e_{t,b} + w1_h . e_{t+1,b} + cb_h
    scores      = (max over valid t of conv) @ out_w.T + out_b

where e = concat(E[tok], U[tok]).  We precompute a fused table

    T[v, 0:64]   = concat(E[v],U[v]) . w0_h        (the "A" half)
    T[v, 64:128] = concat(E[v],U[v]) . w1_h        (the "B" half)

so conv[b,h,t] = T[tok_t, h] + T[tok_{t+1}, 64+h].  The ragged mask is
free: PAD (=1) appears exactly at positions t >= len, so forcing
T[1, 0:64] = -1e30 makes every masked conv position -1e30.

Distribution over 8 cores: phase A (table build) is vocab-sharded
(V/8 rows each) and exchanged with an AllGather; phase B (gather +
conv + masked max + linear head) is batch-sharded (B/8 sentences).
"""

import numpy as np

try:
    import concourse.bass as bass
except ImportError:  # harness runs from a bare directory
    import sys

    sys.path.insert(0, "/opt/trn_rl_repo")
    import concourse.bass as bass

import concourse.bass_isa as bass_isa
import concourse.mybir as mybir
from concourse.bacc import Bacc
import concourse.tile as tile
from concourse.bass_utils import run_bass_kernel_spmd
from concourse.masks import make_identity

V, D, H, S, B = 50000, 300, 64, 512, 256
NCORES = 8
VS = V // NCORES        # vocab rows per core (6250)
BS = B // NCORES        # sentences per core (32)
F = 2 * H               # fused feature width (128)
NEG = -1.0e30
P = 128

F32 = mybir.dt.float32
BF16 = mybir.dt.bfloat16
I32 = mybir.dt.int32


def build_nc(v=V, d=D, h=H, s=S, bs=BS, ncores=NCORES, mm_bf16=True, debug_probe=False, stop_after=None):
    """Build the per-core SPMD Bass program (identical on all cores)."""
    vs = v // ncores
    f = 2 * h
    kb = s // P
    assert s % P == 0 and v % ncores == 0
    fh = h * bs
    n_vt = (vs + P - 1) // P
    chunks = []  # (src_is_u, col0, width) over the 2D concat dim
    for base in range(0, d, P):
        chunks.append((False, base, min(P, d - base)))
    for base in range(0, d, P):
        chunks.append((True, base, min(P, d - base)))
    nch = len(chunks)
    mm_dt = BF16 if mm_bf16 else F32
    t_dt = mm_dt

    nc = Bacc()
    e_sh = nc.dram_tensor("e_shard", [vs, d], F32, kind="ExternalInput")
    u_sh = nc.dram_tensor("u_shard", [vs, d], F32, kind="ExternalInput")
    sent = nc.dram_tensor("sent", [s, bs], I32, kind="ExternalInput")
    sent2 = nc.dram_tensor("sent2", [s, bs], I32, kind="ExternalInput")
    convw = nc.dram_tensor("conv_w", [h, d * 2, 2], F32, kind="ExternalInput")
    convb = nc.dram_tensor("conv_b", [1, h], F32, kind="ExternalInput")
    outw = nc.dram_tensor("out_w", [2, h], F32, kind="ExternalInput")
    outb = nc.dram_tensor("out_b", [1, 2], F32, kind="ExternalInput")
    patch = nc.dram_tensor("patch", [2, f], F32, kind="ExternalInput")
    t_shard = nc.dram_tensor("t_shard", [vs, f], t_dt)
    t_full = nc.dram_tensor("t_full", [v + 1, f], t_dt, addr_space="Shared")
    scores = nc.dram_tensor("scores", [bs, 2], F32, kind="ExternalOutput")
    if debug_probe:
        tsh_out = nc.dram_tensor("tsh_out", [vs, f], F32, kind="ExternalOutput")
        tf_out = nc.dram_tensor("tf_out", [v, f], F32, kind="ExternalOutput")
        ga_out = nc.dram_tensor("ga_out", [P, kb * bs * h], F32, kind="ExternalOutput")
        gb_out = nc.dram_tensor("gb_out", [P, kb * bs * h], F32, kind="ExternalOutput")

    with tile.TileContext(nc) as tc:
        with tc.tile_pool(name="const", bufs=1) as cpool:
            ident = cpool.tile([P, P], F32, tag="identf")
            make_identity(nc, ident[:])
            identb = cpool.tile([P, P], mm_dt, tag="identb")
            if mm_bf16:
                make_identity(nc, identb[:])

            # ---- W2 prep: w2c[ci][dd, k*h + hh] = conv_w[hh, c0+dd, k]
            cw_sb = cpool.tile([h, d * 2 * 2], F32, tag="cw")
            nc.sync.dma_start(
                cw_sb[:], convw[:, :, :].rearrange("a b c -> a (b c)")
            )
            cw_v = cw_sb[:].rearrange("a (b c) -> a b c", c=2)
            w2cs = []
            with tc.tile_pool(name="w2psum", bufs=2, space="PSUM") as wpp:
                for ci, (_, c0, dc) in enumerate(chunks):
                    w2c = cpool.tile([P, f], mm_dt, tag=f"w2_{ci}")
                    w2cs.append(w2c)
                    cglob = c0 + (d if chunks[ci][0] else 0)
                    for k in range(2):
                        tp = wpp.tile([P, h], F32, tag="wtp")
                        nc.tensor.transpose(
                            tp[:dc, :h],
                            cw_v[:, cglob : cglob + dc, k],
                            ident[:h, :h],
                        )
                        nc.vector.tensor_copy(
                            w2c[:dc, k * h : (k + 1) * h], tp[:dc, :h]
                        )

            patch_sb = cpool.tile([2, f], t_dt, tag="patch")
            nc.gpsimd.dma_start(patch_sb[:], patch[:, :])

            # ---- Phase A: T_shard tiles
            with (
                tc.tile_pool(name="pa", bufs=3) as papool,
                tc.tile_pool(name="pa_ps", bufs=3, space="PSUM") as papsum,
                tc.tile_pool(name="pa_acc", bufs=2, space="PSUM") as paacc,
            ):
                for ti in range(n_vt):
                    r0 = ti * P
                    nr = min(P, vs - r0)
                    e_t = papool.tile([P, d], mm_dt, tag="e_t")
                    u_t = papool.tile([P, d], mm_dt, tag="u_t")
                    if mm_bf16:
                        nc.gpsimd.dma_start(e_t[:nr], e_sh[r0 : r0 + nr, :])
                        nc.gpsimd.dma_start(u_t[:nr], u_sh[r0 : r0 + nr, :])
                    else:
                        nc.sync.dma_start(e_t[:nr], e_sh[r0 : r0 + nr, :])
                        nc.sync.dma_start(u_t[:nr], u_sh[r0 : r0 + nr, :])
                    ets = papool.tile([P, nch * P], mm_dt, tag="ets")
                    for ci, (is_u, c0, dc) in enumerate(chunks):
                        src = u_t if is_u else e_t
                        tp = papsum.tile([P, P], mm_dt, tag="tp")
                        nc.tensor.transpose(
                            tp[:dc, :nr],
                            src[:nr, c0 : c0 + dc],
                            identb[:nr, :nr] if mm_bf16 else ident[:nr, :nr],
                        )
                        nc.any.tensor_copy(
                            ets[:dc, ci * P : ci * P + nr], tp[:dc, :nr]
                        )
                    acc = paacc.tile([P, f], F32, tag="acc")
                    for ci, (is_u, c0, dc) in enumerate(chunks):
                        nc.tensor.matmul(
                            acc[:nr, :],
                            lhsT=ets[:dc, ci * P : ci * P + nr],
                            rhs=w2cs[ci][:dc, :],
                            start=(ci == 0),
                            stop=(ci == nch - 1),
                        )
                    t_sb = papool.tile([P, f], t_dt, tag="t_sb")
                    nc.any.tensor_copy(t_sb[:nr], acc[:nr])
                    if ti == 0:
                        # core 0's patch is -1e30 on the A half; others zero
                        nc.vector.tensor_add(
                            t_sb[0:2, :], t_sb[0:2, :], patch_sb[0:2, :]
                        )
                    nc.sync.dma_start(t_shard[r0 : r0 + nr, :], t_sb[:nr])

            # ---- exchange shards
            nc.gpsimd.collective_compute(
                "AllGather",
                mybir.AluOpType.bypass,
                replica_groups=[list(range(ncores))],
                ins=[t_shard[:, :]],
                outs=[t_full[0:v, :]],
            )

            if debug_probe:
                nc.sync.dma_start(tsh_out[:, :], t_shard[:, :])
                nc.sync.dma_start(tf_out[:, :], t_full[:, :])

            neg_sb = cpool.tile([1, f], t_dt, tag="negrow")
            nc.vector.memset(neg_sb[:], NEG)
            nc.sync.dma_start(t_full[v : v + 1, :], neg_sb[:])

            # ---- Phase B: gather + conv + masked max + head
            with (
                tc.tile_pool(name="pb", bufs=1) as pbpool,
                tc.tile_pool(name="pbh", bufs=1) as hpool,
                tc.tile_pool(name="pb_ps", bufs=4, space="PSUM") as pbpsum,
            ):
                # --- token gather via dma_gather (int16 idx; split table)
                # idx order i = b*s + t  ->  out[p=i%128, j=i//128] with
                # j = b*kb + k, t = k*128 + p.
                nidx = s * bs
                nw = nidx // 16
                nj = nidx // P
                split = 32768 if v > 32768 else (v + 1) // 2
                # wrapped idx layout: idx i=b*s+t at (partition i%16, col
                # i//16) = (t%16, b*(s//16)+t//16); the queue-q gather reads
                # partitions [32q, 32q+32), so replicate the wrap into each
                # group a call needs: swa groups 0-3 (queues 0,1), swb
                # groups 4-7 (queues 2,3).
                def load_wrapped(dst, src_dram, groups):
                    for g in groups:
                        nc.sync.dma_start(
                            dst[16 * g : 16 * (g + 1), :],
                            bass.AP(
                                src_dram,
                                0,
                                [[bs, 16], [1, bs], [16 * bs, s // 16]],
                            ),
                        )

                swa = pbpool.tile([P, nw], I32, tag="swa")
                swb = pbpool.tile([P, nw], I32, tag="swb")
                load_wrapped(swa, sent, range(0, 2))
                load_wrapped(swb, sent2, range(0, 2))
                ilo_a = pbpool.tile([P, nw], mybir.dt.int16, tag="ilo_a")
                ihi_a = pbpool.tile([P, nw], mybir.dt.int16, tag="ihi_a")
                ilo_b = pbpool.tile([P, nw], mybir.dt.int16, tag="ilo_b")
                ihi_b = pbpool.tile([P, nw], mybir.dt.int16, tag="ihi_b")
                # queue-0 cores read idxs from partitions [0,16) and [16,32)
                # ilo = tok < split ? tok : 0        (row 0 = -1e30)
                # ihi = tok >= split ? tok-split : v-split   (row v = -1e30)
                for t in (ilo_a, ihi_a, ilo_b, ihi_b):
                    nc.vector.memset(t[:], 0)
                c2 = pbpool.tile([P, nw], I32, tag="c2")
                c1 = pbpool.tile([P, nw], I32, tag="c1")
                d2 = pbpool.tile([P, nw], I32, tag="d2")
                for sw, ilo, ihi in ((swa, ilo_a, ihi_a), (swb, ilo_b, ihi_b)):
                    nc.vector.tensor_scalar(
                        c2[0:32], sw[0:32], split, None, mybir.AluOpType.is_ge
                    )
                    nc.vector.tensor_scalar(
                        c1[0:32], sw[0:32], split, None, mybir.AluOpType.is_lt
                    )
                    nc.vector.tensor_tensor(
                        ilo[0:32], sw[0:32], c1[0:32], op=mybir.AluOpType.mult
                    )
                    nc.vector.tensor_scalar(
                        d2[0:32], sw[0:32], v, None, mybir.AluOpType.subtract
                    )
                    nc.vector.tensor_tensor(
                        d2[0:32], d2[0:32], c2[0:32], op=mybir.AluOpType.mult
                    )
                    nc.vector.tensor_scalar(
                        ihi[0:32], d2[0:32], v - split, None, mybir.AluOpType.add
                    )
                ga = pbpool.tile([P, nj * f], t_dt, tag="ga")
                gah = pbpool.tile([P, nj * f], t_dt, tag="gah")
                gb = pbpool.tile([P, nj * f], t_dt, tag="gb")
                gbh = pbpool.tile([P, nj * f], t_dt, tag="gbh")
                gathers = [
                    (ga, t_full[0 : split, :], ilo_a, 0),
                    (gah, t_full[split : v + 1, :], ihi_a, 0),
                    (gb, t_full[0 : split, :], ilo_b, 0),
                    (gbh, t_full[split : v + 1, :], ihi_b, 0),
                ]
                # ring carveout holds 2048 descs/direction; one call may
                # carry at most ~16k idxs (descs = nidx/16 + 1), so chunk.
                max_chunk = 8192
                chunks_i = []
                i0 = 0
                while i0 < nidx:
                    cn = min(max_chunk, nidx - i0)
                    chunks_i.append((i0, cn))
                    i0 += cn
                for out_t, in_ap, idx_t, q in gathers:
                    ov = out_t[:].rearrange("p (j c) -> p j c", c=f)
                    for i0, cn in chunks_i:
                        nc.gpsimd.dma_gather(
                            out_ap=ov[:, i0 // P : (i0 + cn) // P, :],
                            in_ap=in_ap,
                            idxs_ap=idx_t[:, i0 // 16 : (i0 + cn) // 16],
                            num_idxs=cn,
                            num_idxs_reg=cn,
                            elem_size=f,
                            elem_step=f,
                            queue_num=q,
                            single_packet=False,
                        )
                # merge: wrong-table entries are -1e30, so max picks
                # the real row
                nc.any.tensor_max(ga[:], ga[:], gah[:])
                nc.any.tensor_max(gb[:], gb[:], gbh[:])
                # conv[p, (b, k, c)] = ga.Ahalf + gb.Bhalf
                conv = pbpool.tile([P, nj * h], F32, tag="conv")
                gav = ga[:].rearrange("p (j c) -> p j c", c=f)
                gbv = gb[:].rearrange("p (j c) -> p j c", c=f)
                nc.any.tensor_add(
                    conv[:].rearrange("p (j c) -> p j c", c=h),
                    gav[:, :, 0:h],
                    gbv[:, :, h:f],
                )
                c4 = conv[:].rearrange("p (b k c) -> p b k c", b=bs, k=kb)
                # max over k blocks -> m [p, (b, h)]
                if kb > 1:
                    m = hpool.tile([P, fh], F32, tag="m")
                    nc.any.tensor_max(m[:], c4[:, :, 0, :], c4[:, :, 1, :])
                    for k in range(2, kb):
                        nc.any.tensor_max(m[:], m[:], c4[:, :, k, :])
                    m_ap = m[:]
                else:
                    m_ap = c4[:, :, 0, :]
                # per-sentence: transpose [128 tok, 64 feat] -> [64, 128] and
                # reduce over the 128 tokens, into pooled_t[:, b]
                pooled_t = pbpool.tile([h + 1, bs], F32, tag="pooled_t")
                nc.vector.memset(pooled_t[h : h + 1, :], 1.0)
                for b in range(bs):
                    mt = pbpsum.tile([h, P], F32, tag="mt")
                    msl = (
                        m[:, b * h : (b + 1) * h]
                        if kb > 1
                        else c4[:, b, 0, :]
                    )
                    nc.tensor.transpose(mt[:, :], msl, ident[:, :])
                    nc.vector.reduce_max(
                        pooled_t[0:h, b : b + 1],
                        mt[:, :],
                        axis=mybir.AxisListType.X,
                    )
                cb_t = pbpool.tile([h, 1], F32, tag="cb_t")
                nc.sync.dma_start(cb_t[:, :], convb[:, :].rearrange("o c -> c o"))
                nc.vector.tensor_scalar_add(
                    pooled_t[0:h, :], pooled_t[0:h, :], cb_t[:, :]
                )
                ow_t = pbpool.tile([h + 1, 2], F32, tag="ow_t")
                nc.sync.dma_start(ow_t[0:h, :], outw[:, :].rearrange("a c -> c a"))
                nc.sync.dma_start(ow_t[h : h + 1, :], outb[:, :])
                sc_ps = pbpsum.tile([bs, 2], F32, tag="sc")
                nc.tensor.matmul(
                    sc_ps[:, :],
                    lhsT=pooled_t[:, :],
                    rhs=ow_t[:, :],
                    start=True,
                    stop=True,
                )
                sc_sb = pbpool.tile([bs, 2], F32, tag="sc_sb")
                nc.vector.tensor_copy(sc_sb[:], sc_ps[:])
                nc.sync.dma_start(scores[:, :], sc_sb[:])

    nc.finalize()
    return nc


_NC_CACHE = {}


def _get_nc():
    if "nc" not in _NC_CACHE:
        _NC_CACHE["nc"] = build_nc()
    return _NC_CACHE["nc"]


def make_in_maps(sentences, E, U, conv_w, conv_b, out_w, out_b,
                 v=V, h=H, ncores=NCORES):
    vs = v // ncores
    bs = sentences.shape[1] // ncores
    f = 2 * h
    sent_shift = np.concatenate(
        [sentences[1:], np.zeros((1, sentences.shape[1]), np.int32)], axis=0
    )
    in_maps = []
    for c in range(ncores):
        pt = np.zeros((2, f), np.float32)
        if c == 0:
            pt[0, :] = NEG
            pt[1, :h] = NEG
        in_maps.append(
            {
                "e_shard": np.ascontiguousarray(E[c * vs : (c + 1) * vs]),
                "u_shard": np.ascontiguousarray(U[c * vs : (c + 1) * vs]),
                "sent": np.ascontiguousarray(
                    sentences[:, c * bs : (c + 1) * bs]
                ),
                "sent2": np.ascontiguousarray(
                    sent_shift[:, c * bs : (c + 1) * bs]
                ),
                "conv_w": conv_w,
                "conv_b": conv_b.reshape(1, h),
                "out_w": out_w,
                "out_b": out_b.reshape(1, 2),
                "patch": pt,
            }
        )
    return in_maps


def kernel(sentences, E, U, conv_w, conv_b, out_w, out_b):
    sentences = np.asarray(sentences, dtype=np.int32)
    E = np.asarray(E, dtype=np.float32)
    U = np.asarray(U, dtype=np.float32)
    conv_w = np.asarray(conv_w, dtype=np.float32)
    conv_b = np.asarray(conv_b, dtype=np.float32)
    out_w = np.asarray(out_w, dtype=np.float32)
    out_b = np.asarray(out_b, dtype=np.float32)

    nc = _get_nc()
    in_maps = make_in_maps(sentences, E, U, conv_w, conv_b, out_w, out_b)
    res = run_bass_kernel_spmd(nc, in_maps, list(range(NCORES)))
    return np.concatenate(
        [res.results[c]["scores"] for c in range(NCORES)], axis=0
    )
